# revision 53
# baseline (speedup 1.0000x reference)
"""Trainium2 Bass kernel for MixActivConv2d (mixed-precision fake-quant + 1x1 conv).

Reference computation:
  sel = x[:, ch]                                   # gather 8 channels
  activ = sum_i softmax(aa)[i] * uq(sel, bit_i)    # global-minmax fake quant
  x_q = x with sel channels replaced by activ
  w_q = sum_i softmax(aw)[i] * uq(w, bit_i)
  out = conv1x1(x_q, w_q)  ==  w_q[256,256] @ x_q[b, 256, 4096]

Strategy (8 cores, data-parallel over batch, 4 batches/core):
  - out[b] = Wq @ x[b] + WqselT.T @ (activ - sel)[b]   (rank-8 correction,
    so the streamed x tiles never need a scatter)
  - matmul operands fp16 (1 PE cycle/row vs 4 for fp32); the x stream and
    the replicated sel minmax copy are fp16
  - output: PSUM -> DRAM fp32 DMAs issued round-robin on all four DMA
    queues (the cost model serializes transfers per ISSUING queue, and
    queues run concurrently - so spreading is the whole game; it also
    kills 64 engine evict-copies)
  - quant BUCKETING stays exact: selloc + scale-constant chain in fp32;
    min/max over the fp16 sel copy is exact over the rounded values
  - correction lhsT: host tiles W[:,ch]^T down all 128 partitions;
    quantize once, 8 masked variants via ACT activation(scale=0/1 mask)
    (exact multiply, fp16 cast folded) - no partition-moving DMAs
  - engine split: DVE = W minmax partials + both const chains + sel min
    partials + delta half, Pool = sel max (all-axis) + W/corr quant +
    delta half, ACT = variant masking + casts, SP = x stream; out writes
    rotate across ACT/SP/Pool/DVE queues
  - minmax results land replicated via partition_all_reduce so const
    chains run 128-wide (no partition_broadcast on the critical path);
    an explicit dep keeps the serial W-consts chain from being stretched
    by 2.2us sel reduce passes interleaving on DVE
"""

import sys
from contextlib import ExitStack

import numpy as np

sys.path.insert(0, "/opt/trn_rl_repo")

import concourse.bass as bass  # noqa: E402
import concourse.bass_isa as bass_isa  # noqa: E402
import concourse.mybir as mybir  # noqa: E402
import concourse.tile as tile  # noqa: E402
from concourse import bacc  # noqa: E402
from concourse.tile import add_dep_helper  # noqa: E402

NCORES = 8
B, C, H, W = 32, 256, 64, 64
HW = H * W  # 4096
BPC = B // NCORES  # batches per core = 4
NSEL = 8
QMAX = (3.0, 15.0, 255.0)  # 2^bit - 1 for bits (2, 4, 8)
MAGIC = 12582912.0  # 1.5 * 2**23: x + MAGIC - MAGIC == rne-round(x) for |x| < 2^22
F32 = mybir.dt.float32
F16 = mybir.dt.float16
ALU = mybir.AluOpType
AXIS = mybir.AxisListType
ACTF = mybir.ActivationFunctionType

# fp16 sel minmax copy (2 MB DMA) measures ~1.1e-2 rel err vs the 2e-2 gate;
# flip to False (4 MB fp32, exact scales, ~2e-3 err) if the margin shrinks.
SELRED_F16 = True
SELRED_DT = F16 if SELRED_F16 else F32


def _emit_scalar_consts(nc, vals, scal_mx, scal_mn, sw, tmp, d3, y3, nparts=1, eng=None):
    """Scale-const chain, replicated across `nparts` partitions. Writes vals
    [nparts,10]: cols 0..2 inv_i (=1/scale_i), 3..5 k_i (=sw_i*scale_i),
    6 mn, 7 MAGIC. Returns the list of final instructions (for dep hooks).

    scale_i = fl((mx-mn) * fl(1/qmax_i)): within ~1.5 ulp of the reference
    IEEE division; measured on the fixed inputs, the induced bucket flips
    change the final max rel err by < 1e-6 (the fp16 sel-minmax shift
    dominates), so the exact-residual Newton chain is not worth its 17
    serial steps on the critical path.
    """

    eng = eng if eng is not None else nc.vector
    P = nparts
    scale3 = tmp[0:P, 0:3]
    rng = tmp[0:P, 6:7]
    eng.tensor_sub(rng, scal_mx, scal_mn)
    eng.tensor_mul(scale3, rng.to_broadcast((P, 3)), y3)
    i1_ = nc.vector.reciprocal(vals[0:P, 0:3], scale3)
    i2 = eng.tensor_mul(vals[0:P, 3:6], scale3, sw)
    i3 = eng.tensor_copy(vals[0:P, 6:7], scal_mn)
    i4 = eng.memset(vals[0:P, 7:8], MAGIC)
    return [i1_, i2, i3, i4]


def _emit_quant(nc, pool, src, cbuf, nparts, nfree, out=None, sub_src=False, eng=None, sfx="", u_pre=None):
    """Emit the 3-bit blended fake-quant of src [nparts, nfree].

    u = src - mn
    r_i = u*inv_i + MAGIC          (two-stage TS op; the add rounds to
                                    integer RNE per ALU stage)
    p_i = (r_i - MAGIC) * k_i      (subtract exact, result = round(u/s)*k)
    result = p0 + p1 + p2 + mn     [- src if sub_src, giving the delta]
    Returns (output tile, instruction list).
    """
    eng = eng if eng is not None else nc.vector
    insts = []
    if u_pre is not None:
        u = u_pre
    else:
        u = pool.tile([nparts, nfree], F32, tag=f"qu_{nparts}_{nfree}{sfx}", name="qu")
        insts.append(eng.tensor_scalar(u, src, cbuf[:, 6:7], None, op0=ALU.subtract))
    p = []
    for i in range(3):
        # separate mul/add ops (not fused ACT): per-op IEEE fp32 rounding
        # must match the reference's separate ops, else near-tie elements
        # flip into the next quant bucket on HW
        pi = pool.tile(
            [nparts, nfree], F32, tag=f"ptmp{i}_{nparts}_{nfree}{sfx}", name=f"ptmp{i}"
        )
        insts.append(eng.tensor_scalar(
            pi, u, cbuf[:, i : i + 1], MAGIC, op0=ALU.mult, op1=ALU.add
        ))
        insts.append(eng.tensor_scalar(
            pi, pi, MAGIC, cbuf[:, 3 + i : 4 + i], op0=ALU.subtract, op1=ALU.mult
        ))
        p.append(pi)
    insts.append(eng.tensor_add(p[0], p[0], p[1]))
    insts.append(eng.tensor_add(p[0], p[0], p[2]))
    outt = out if out is not None else pool.tile(
        [nparts, nfree], F32, tag=f"qout_{nparts}_{nfree}{sfx}", name="qout"
    )
    if sub_src:
        # delta = (acc + mn) - src  (STT has no POOL opcode: always DVE)
        insts.append(nc.vector.scalar_tensor_tensor(
            outt, p[0], cbuf[:, 6:7], src, op0=ALU.add, op1=ALU.subtract
        ))
    else:
        insts.append(eng.tensor_scalar(outt, p[0], cbuf[:, 6:7], None, op0=ALU.add))
    return outt, insts


def _kernel_body(ctx, tc, ch, x_ap, selred_ap, selloc_ap, ws_ap, al_ap, out_ap, reps=1):
    nc = tc.nc

    const = ctx.enter_context(tc.tile_pool(name="const", bufs=1))
    rhs_pool = ctx.enter_context(tc.tile_pool(name="rhs", bufs=4))
    out_pool = ctx.enter_context(tc.tile_pool(name="outsb", bufs=3))
    psB = ctx.enter_context(tc.tile_pool(name="psB", bufs=4, space="PSUM"))

    # ---- inputs. selred chunks 0-2 lead the ACT queue, chunk 3 rides the
    # SP head so all four land by ~6us; the x stream follows on SP. ----
    alphas = const.tile([1, 6], F32)
    nc.gpsimd.dma_start(alphas[:], al_ap)  # SWDGE: off both HWDGE streams
    selredc = [
        const.tile([128, 2048], SELRED_DT, name=f"selredc{i}", tag=f"selredc{i}")
        for i in range(4)
    ]
    # W arrives pre-transposed from the host (quantization is elementwise,
    # so quant(W^T) == quant(W)^T): the quantized tiles ARE the lhsT
    # operands — no PE transposes, no PSUM staging, no identity matrix.
    # wcomb = [W^T k0 | W^T k1 | tiled W[:,ch]^T] in one [128,768] tile so
    # the whole weights path is one u-op + one split quant chain.
    wcomb = const.tile([128, 2 * C], F32)
    nc.sync.dma_start(wcomb[:], ws_ap)
    wtside = wcomb[:, 0 : 2 * C]
    nc.sync.dma_start(selredc[3][:], selred_ap[:, 3 * 2048 : 4 * 2048])
    for i in range(3):
        nc.scalar.dma_start(selredc[i][:], selred_ap[:, i * 2048 : (i + 1) * 2048])
    selloc = const.tile([128, 1024], F32)
    nc.scalar.dma_start(selloc[:], selloc_ap)

    # staging tile for the combined sel all-reduce: col0 = -min, col1 = max
    # (partition 0 holds the real max, rest -1e30)
    stage = const.tile([128, 2], F32)
    nc.gpsimd.memset(stage[:, 1:2], -1e30)

    with tc.high_priority():
        # ---- softmax of both alpha vectors (on partition 0), then one
        # early broadcast so both const chains can run 128-wide ----
        ex = const.tile([1, 6], F32)
        nc.scalar.activation(ex[:], alphas[:], ACTF.Exp)
        sums = const.tile([1, 8], F32)
        nc.vector.tensor_reduce(sums[0:1, 0:1], ex[0:1, 0:3], axis=AXIS.X, op=ALU.add)
        nc.vector.tensor_reduce(sums[0:1, 1:2], ex[0:1, 3:6], axis=AXIS.X, op=ALU.add)
        nc.vector.reciprocal(sums[0:1, 2:3], sums[0:1, 0:1])
        nc.vector.reciprocal(sums[0:1, 3:4], sums[0:1, 1:2])
        sw1 = const.tile([1, 6], F32)  # cols 0..2 = sw_activ, 3..5 = sw_weight
        nc.vector.tensor_scalar(sw1[0:1, 0:3], ex[0:1, 0:3], sums[0:1, 2:3], None, op0=ALU.mult)
        nc.vector.tensor_scalar(
            sw1[0:1, 3:6], ex[0:1, 3:6], sums[0:1, 3:4], None, op0=ALU.mult
        )
        sw = const.tile([128, 6], F32)
        nc.gpsimd.partition_broadcast(sw[:], sw1[0:1, :])

        # qmax and fl(1/qmax) constant vectors, replicated
        d3 = const.tile([128, 3], F32)
        y3 = const.tile([128, 3], F32)
        for i, qm in enumerate(QMAX):
            nc.gpsimd.memset(d3[:, i : i + 1], float(qm))
            nc.gpsimd.memset(y3[:, i : i + 1], float(np.float32(1.0) / np.float32(qm)))

        # ---- W min/max: DVE free-axis partials (runs before the sel data
        # lands), then a Pool all-reduce -> replicated scalars. The Pool
        # maxes are forced to wait for this all-reduce so the W-consts
        # chain starts early. ----
        wpart = const.tile([128, 2], F32)
        nc.vector.tensor_reduce(wpart[:, 0:1], wcomb[:, 0 : 2 * C], axis=AXIS.X, op=ALU.max)
        wminp = const.tile([128, 1], F32)
        nc.vector.tensor_reduce(wminp[:], wcomb[:, 0 : 2 * C], axis=AXIS.X, op=ALU.min)
        wneg_inst = nc.vector.tensor_scalar(wpart[:, 1:2], wminp[:], -1.0, None, op0=ALU.mult)
        gredw = const.tile([128, 2], F32)  # col0 wmx, col1 -wmn (replicated)
        arw_inst = nc.gpsimd.partition_all_reduce(
            gredw[:, 0:2], wpart[:, 0:2], channels=128, reduce_op=bass_isa.ReduceOp.max
        )
        wmn = const.tile([128, 1], F32)
        nc.vector.tensor_scalar(wmn[:], gredw[:, 1:2], -1.0, None, op0=ALU.mult)

        # ---- W consts (replicated on DVE), then one split quant chain:
        # Pool takes the lhsT 512 cols, ACT takes the 256 corr cols (its
        # fused scale*x+bias rounding differs from the reference only for
        # elements within one fp32 ulp of a .5 boundary - a few flips at
        # ~1e-3 rel each, inside the error budget) ----
        cbufw = const.tile([128, 10], F32)
        tmpw = const.tile([128, 40], F32)
        wconst_tail = _emit_scalar_consts(
            nc, cbufw, gredw[:, 0:1], wmn[:, 0:1], sw[:, 3:6], tmpw, d3, y3,
            nparts=128, eng=nc.vector,
        )
        uw = const.tile([128, 2 * C], F32)
        nc.gpsimd.tensor_scalar(uw[:], wcomb[:, 0 : 2 * C], cbufw[:, 6:7], None, op0=ALU.subtract)
        lhsT32 = const.tile([128, 2 * C], F32)
        lhsT = const.tile([128, 2 * C], F16)
        wq_chains = {}
        for mh in range(2):  # m0 cols on Pool (early); m1 cols on DVE (late,
            for k in range(2):  # after its activ half -- keeps Pool's queue
                c0 = k * C + mh * 128  # clear for wave drains)
                _, wq_chains[(mh, k)] = _emit_quant(
                    nc, const, wcomb[:, c0 : c0 + 128], cbufw, 128, 128,
                    out=lhsT32[:, c0 : c0 + 128],
                    eng=nc.gpsimd if mh == 0 else nc.vector, sfx=f"wq{mh}{k}",
                    u_pre=uw[:, c0 : c0 + 128],
                )
                nc.scalar.copy(lhsT[:, c0 : c0 + 128], lhsT32[:, c0 : c0 + 128])

    # ---- sel min/max: DVE takes the 4 min passes (plain min partials),
    # Pool takes the 4 all-axis max passes; one partition_all_reduce
    # replicates both so the consts chain runs 128-wide. The min passes
    # explicitly wait for the W consts chain (else the static schedule
    # interleaves 2.2us passes between its serial 60ns steps). ----
    sminp = const.tile([128, 4], F32)  # min partials
    sminc = const.tile([128, 1], F32)
    pmax = const.tile([1, 4], F32)  # per-chunk global maxes (partition 0)
    for i in range(4):
        mininst = nc.vector.tensor_reduce(
            sminp[:, i : i + 1], selredc[i][:], axis=AXIS.X, op=ALU.min
        )
        if i == 0:
            # both W minmax partials go first: they unlock AR_w -> W consts
            add_dep_helper(mininst.ins, wneg_inst.ins, reason="W partials first")
        maxinst = nc.gpsimd.tensor_reduce(
            pmax[0:1, i : i + 1], selredc[i][:], axis=AXIS.XYZWC, op=ALU.max
        )
        if i == 1:
            add_dep_helper(maxinst.ins, arw_inst.ins, reason="AR_w before c1max")
        if i == 2:
            lastmin_inst = mininst

    nc.vector.tensor_reduce(sminc[:], sminp[:, 0:4], axis=AXIS.X, op=ALU.min)
    nc.vector.tensor_scalar(stage[:, 0:1], sminc[:], -1.0, None, op0=ALU.mult)
    nc.vector.tensor_reduce(stage[0:1, 1:2], pmax[0:1, 0:4], axis=AXIS.X, op=ALU.max)
    gred = const.tile([128, 2], F32)  # col0 -smn, col1 smx (replicated)
    nc.gpsimd.partition_all_reduce(
        gred[:, 0:2], stage[:, 0:2], channels=128, reduce_op=bass_isa.ReduceOp.max
    )

    smn = const.tile([128, 1], F32)
    nc.vector.tensor_scalar(smn[:], gred[:, 0:1], -1.0, None, op0=ALU.mult)

    # ---- sel consts (replicated) + activ = blended fake-quant of sel;
    # cast to fp16 and scatter into the rhs k1 tiles (rows 120-127) ----
    cbufs = const.tile([128, 10], F32)
    tmps = const.tile([128, 40], F32)
    _emit_scalar_consts(
        nc, cbufs, gred[:, 1:2], smn[:, 0:1], sw[:, 0:3], tmps, d3, y3,
        nparts=128, eng=nc.vector,
    )
    activ16 = const.tile([128, 1024], F16)
    # quarter-split across DVE and Pool so the r=0 half (which gates the
    # first k1 matmuls via the scatter) finishes ~1us sooner; the final
    # +mn op writes fp16 directly (the scatter is the only consumer)
    _, ac_a = _emit_quant(
        nc, const, selloc[:, 0:256], cbufs, 128, 256,
        out=activ16[:, 0:256], eng=nc.vector, sfx="sa1",
    )
    _emit_quant(
        nc, const, selloc[:, 256:512], cbufs, 128, 256,
        out=activ16[:, 256:512], eng=nc.gpsimd, sfx="sa2",
    )
    _, ac_b = _emit_quant(
        nc, const, selloc[:, 512:768], cbufs, 128, 256,
        out=activ16[:, 512:768], eng=nc.vector, sfx="sb1",
    )
    _emit_quant(
        nc, const, selloc[:, 768:1024], cbufs, 128, 256,
        out=activ16[:, 768:1024], eng=nc.gpsimd, sfx="sb2",
    )
    # the m1 weight chains run after DVE's activ work (PE's m1 waves are
    # ~15us out; the activations gate every k1 main matmul)
    for k in range(2):
        add_dep_helper(wq_chains[(1, k)][0].ins, ac_b[-1].ins,
                       reason="activ before Wq-m1 on DVE")

    # ---- main loop. Scatter the quantized activations over the fp16 sel
    # rows of each k1 tile (selloc column r*512+s holds pixel r*2048+q*512+s,
    # so src/dst are clean 3D APs), then per (m-half, batch) wave: fill all
    # 8 PSUM banks with the K=256 mains, evict pairs to fp16 SBUF and DMA
    # out contiguous 2KB-per-partition chunks. All m=0 waves run before
    # m=1 so the m1 weight chains can quantize late without stalling PE. ----
    rhs = []
    for b in range(BPC):
        rhs0 = rhs_pool.tile([128, HW], F16, tag="rhs0")
        nc.sync.dma_start(rhs0[:], x_ap[b, 0:128, :])
        rhs1 = rhs_pool.tile([128, HW], F16, tag="rhs1")
        nc.sync.dma_start(rhs1[:], x_ap[b, 128:256, :])
        rhs.append((rhs0, rhs1))
    # pixel p = r*2048 + q*512 + s, so the r-half scatters are plain 2D
    # slices; gA's k1 matmuls (pixels 0:2048) only need the r=0 half, which
    # the DVE activ chain finishes first
    for b in range(BPC):
        for r in range(2):
            sc_eng = [nc.scalar, nc.gpsimd] if b < 2 else [nc.gpsimd, nc.gpsimd]
            for q in range(4):
                sc_eng[q % len(sc_eng)].dma_start(
                    rhs[b][1][120:128,
                              r * 2048 + q * 512 : r * 2048 + (q + 1) * 512],
                    activ16[b * 32 + q * 8 : b * 32 + (q + 1) * 8,
                            r * 512 : (r + 1) * 512],
                )

    # GPSIMD cannot access PSUM (BIR verifier) -> evicts on ACT/DVE only;
    # Pool carries extra out-writes instead
    ev_seq = ["D", "A", "D", "D", "A", "D", "A", "D"] + ["D", "A"] * 28
    wq_seq = ["A", "P", "A", "P", "A", "P", "P", "A",
              "S", "P", "S", "A", "P", "S", "P", "S",
              "A", "P", "S", "P", "S", "A", "P", "S",
              "P", "P", "S", "P"]
    ev_map = {"D": nc.vector.tensor_copy, "A": nc.scalar.copy}
    wq_map = {"S": nc.sync, "A": nc.scalar, "P": nc.gpsimd}
    ev_i = 0
    wq_i = 0
    # ---- PE clock warmup: ~20 discarded matmuls (into wave 1's first
    # PSUM pair, overwritten by its real start=True mains) keep the PE
    # busy from ~12us so the real stream runs at full clock with no ramp.
    # Gated on the last sel min pass so they don't run at t=0 and idle out.
    pair00 = psB.tile([128, 1024], F32, name="ptile", tag="ptile")
    for wi in range(14):
        wmm = nc.tensor.matmul(
            pair00[:, 0:512],
            selredc[0][:, 0:128],
            selredc[0][:, 512:1024],
            start=True,
            stop=True,
            skip_group_check=True,
        )
        if wi == 0:
            add_dep_helper(wmm.ins, lastmin_inst.ins, reason="warmup after mins")
    first_mm = [None]
    for rep in range(reps):
        for m in range(2):
            for b in range(BPC):
                rhs0, rhs1 = rhs[b]
                groups = ([0, 1, 2, 3], [4, 5, 6, 7])  # contiguous pixels
                # [128,1024] PSUM tiles (2 banks each): matmuls hit 512-wide
                # slices, the evict reads the pair in one op
                pairs = {}
                for ns in groups:
                    for n in ns:
                        if n % 2 == 0:
                            if rep == 0 and m == 0 and b == 0 and n == 0:
                                pairs[0] = pair00
                            else:
                                pairs[n // 2] = psB.tile(
                                    [128, 1024], F32, name="ptile", tag="ptile"
                                )
                        mm = nc.tensor.matmul(
                            pairs[n // 2][:, (n % 2) * 512 : (n % 2 + 1) * 512],
                            lhsT[:, m * 128 : (m + 1) * 128],
                            rhs0[:, n * 512 : (n + 1) * 512],
                            start=True,
                            stop=False,
                        )

                    for n in ns:
                        nc.tensor.matmul(
                            pairs[n // 2][:, (n % 2) * 512 : (n % 2 + 1) * 512],
                            lhsT[:, C + m * 128 : C + (m + 1) * 128],
                            rhs1[:, n * 512 : (n + 1) * 512],
                            start=False,
                            stop=True,
                        )
                outsb = out_pool.tile([128, HW], F16, name="outsb", tag="outsb")
                is_last = m == 1 and b >= BPC - 2 and rep == reps - 1
                for gi, ns in enumerate(groups):
                    if is_last:
                        # final drain: per-512 chunks fanned across engines
                        # and queues so the tail after the last matmul is
                        # one small copy + one small DMA
                        levs = [nc.scalar.copy, nc.vector.tensor_copy,
                                nc.scalar.copy, nc.vector.tensor_copy]
                        lwqs = ([nc.scalar, nc.sync, nc.gpsimd, nc.sync]
                                if b == BPC - 1 else
                                [nc.gpsimd, nc.sync, nc.scalar, nc.gpsimd])
                        for j, n in enumerate(ns):
                            levs[j](
                                outsb[:, n * 512 : (n + 1) * 512],
                                pairs[n // 2][:, (n % 2) * 512 : (n % 2 + 1) * 512],
                            )
                            lwqs[j].dma_start(
                                out_ap[b, m * 128 : (m + 1) * 128,
                                       n * 512 : (n + 1) * 512],
                                outsb[:, n * 512 : (n + 1) * 512],
                            )
                        continue
                    # paired evicts (2 x [128,1024]), each written out as
                    # soon as it lands (pipelines the drain within a group)
                    for half in range(2):
                        pr = ns[0] // 2 + half
                        ev_map[ev_seq[ev_i % len(ev_seq)]](
                            outsb[:, pr * 1024 : (pr + 1) * 1024], pairs[pr][:]
                        )
                        ev_i += 1
                        wq_map[wq_seq[wq_i % len(wq_seq)]].dma_start(
                            out_ap[b, m * 128 : (m + 1) * 128,
                                   pr * 1024 : (pr + 1) * 1024],
                            outsb[:, pr * 1024 : (pr + 1) * 1024],
                        )
                        wq_i += 1


def build_program(ch, reps=1):
    nc = bacc.Bacc(
        "TRN2", target_bir_lowering=False, debug=False, num_devices=NCORES
    )
    x_t = nc.dram_tensor("x", [BPC, C, HW], F16, kind="ExternalInput").ap()
    selred_t = nc.dram_tensor("selred", [128, 8192], SELRED_DT, kind="ExternalInput").ap()
    selloc_t = nc.dram_tensor("selloc", [128, 1024], F32, kind="ExternalInput").ap()
    ws_t = nc.dram_tensor("wselt", [128, 2 * C], F32, kind="ExternalInput").ap()
    al_t = nc.dram_tensor("alphas", [1, 6], F32, kind="ExternalInput").ap()
    out_t = nc.dram_tensor("out", [BPC, C, HW], F16, kind="ExternalOutput").ap()
    with tile.TileContext(nc) as tc:
        with ExitStack() as ctx:
            _kernel_body(
                ctx, tc, ch, x_t, selred_t, selloc_t, ws_t, al_t, out_t,
                reps=reps,
            )
    nc.compile()
    return nc


def make_in_maps(x, alpha_activ, alpha_weight, conv_weight, selected_channels):
    x = np.ascontiguousarray(np.asarray(x, dtype=np.float32).reshape(B, C, HW))
    ch = [int(v) for v in np.asarray(selected_channels).ravel()]
    sel = np.ascontiguousarray(x[:, ch, :])  # [32, 8, 4096]
    # channel permutation: the 8 selected channels go LAST (k1 rows 120-127)
    # so the quantized activations scatter into the rhs tiles as one
    # contiguous partition block; permuting x's channels and W's columns
    # identically leaves the conv output unchanged
    perm = [c for c in range(C) if c not in set(ch)] + ch
    selred_np = np.float16 if SELRED_F16 else np.float32
    selred = sel.reshape(128, 8192).astype(selred_np)
    x16 = x[:, perm, :].astype(np.float16)
    alphas = np.concatenate(
        [np.asarray(alpha_activ).ravel(), np.asarray(alpha_weight).ravel()]
    ).astype(np.float32).reshape(1, 6)
    wmat = np.asarray(conv_weight, dtype=np.float32).reshape(C, C)
    wt = wmat.T[perm, :]  # rows follow the channel permutation
    # cols 0:512 = permuted W^T k-chunks side by side
    wselt = np.concatenate([wt[0:128, :], wt[128:256, :]], axis=1).astype(np.float32)
    wselt = np.ascontiguousarray(wselt)
    in_maps = []
    for c in range(NCORES):
        xs = np.ascontiguousarray(x16[c * BPC : (c + 1) * BPC])
        # selloc layout: partition p = b*32 + q*8 + j, col r*512+s holds
        # sel[core*4+b, j, r*2048 + q*512 + s] -- so PSUM group g covers the
        # contiguous pixel range [g*2048, (g+1)*2048) yet needs only delta
        # column-half g
        sl = sel[c * BPC : (c + 1) * BPC].reshape(BPC, NSEL, 2, 4, 512)
        selloc = np.ascontiguousarray(
            sl.transpose(0, 3, 1, 2, 4).reshape(128, 1024)
        )
        in_maps.append(
            {
                "x": xs,
                "selred": selred,
                "selloc": selloc,
                "wselt": wselt,
                "alphas": alphas,
            }
        )
    return ch, in_maps


def kernel(x, alpha_activ, alpha_weight, conv_weight, selected_channels):
    from concourse.bass_utils import run_bass_kernel_spmd

    ch, in_maps = make_in_maps(
        x, alpha_activ, alpha_weight, conv_weight, selected_channels
    )
    nc = build_program(ch)
    res = run_bass_kernel_spmd(nc, in_maps, core_ids=list(range(NCORES)))
    outs = [
        res.results[c]["out"].astype(np.float32).reshape(BPC, C, H, W)
        for c in range(NCORES)
    ]
    return np.concatenate(outs, axis=0)


# revision 57
# speedup vs baseline: 1.0029x; 1.0029x over previous
"""Trainium2 Bass kernel for MixActivConv2d (mixed-precision fake-quant + 1x1 conv).

Reference computation:
  sel = x[:, ch]                                   # gather 8 channels
  activ = sum_i softmax(aa)[i] * uq(sel, bit_i)    # global-minmax fake quant
  x_q = x with sel channels replaced by activ
  w_q = sum_i softmax(aw)[i] * uq(w, bit_i)
  out = conv1x1(x_q, w_q)  ==  w_q[256,256] @ x_q[b, 256, 4096]

Strategy (8 cores, data-parallel over batch, 4 batches/core):
  - out[b] = Wq @ x[b] + WqselT.T @ (activ - sel)[b]   (rank-8 correction,
    so the streamed x tiles never need a scatter)
  - matmul operands fp16 (1 PE cycle/row vs 4 for fp32); the x stream and
    the replicated sel minmax copy are fp16
  - output: PSUM -> DRAM fp32 DMAs issued round-robin on all four DMA
    queues (the cost model serializes transfers per ISSUING queue, and
    queues run concurrently - so spreading is the whole game; it also
    kills 64 engine evict-copies)
  - quant BUCKETING stays exact: selloc + scale-constant chain in fp32;
    min/max over the fp16 sel copy is exact over the rounded values
  - correction lhsT: host tiles W[:,ch]^T down all 128 partitions;
    quantize once, 8 masked variants via ACT activation(scale=0/1 mask)
    (exact multiply, fp16 cast folded) - no partition-moving DMAs
  - engine split: DVE = W minmax partials + both const chains + sel min
    partials + delta half, Pool = sel max (all-axis) + W/corr quant +
    delta half, ACT = variant masking + casts, SP = x stream; out writes
    rotate across ACT/SP/Pool/DVE queues
  - minmax results land replicated via partition_all_reduce so const
    chains run 128-wide (no partition_broadcast on the critical path);
    an explicit dep keeps the serial W-consts chain from being stretched
    by 2.2us sel reduce passes interleaving on DVE
"""

import sys
from contextlib import ExitStack

import numpy as np

sys.path.insert(0, "/opt/trn_rl_repo")

import concourse.bass as bass  # noqa: E402
import concourse.bass_isa as bass_isa  # noqa: E402
import concourse.mybir as mybir  # noqa: E402
import concourse.tile as tile  # noqa: E402
from concourse import bacc  # noqa: E402
from concourse.tile import add_dep_helper  # noqa: E402

NCORES = 8
B, C, H, W = 32, 256, 64, 64
HW = H * W  # 4096
BPC = B // NCORES  # batches per core = 4
NSEL = 8
QMAX = (3.0, 15.0, 255.0)  # 2^bit - 1 for bits (2, 4, 8)
MAGIC = 12582912.0  # 1.5 * 2**23: x + MAGIC - MAGIC == rne-round(x) for |x| < 2^22
F32 = mybir.dt.float32
F16 = mybir.dt.float16
ALU = mybir.AluOpType
AXIS = mybir.AxisListType
ACTF = mybir.ActivationFunctionType

# fp16 sel minmax copy (2 MB DMA) measures ~1.1e-2 rel err vs the 2e-2 gate;
# flip to False (4 MB fp32, exact scales, ~2e-3 err) if the margin shrinks.
SELRED_F16 = True
SELRED_DT = F16 if SELRED_F16 else F32


def _emit_scalar_consts(nc, vals, scal_mx, scal_mn, sw, tmp, d3, y3, nparts=1, eng=None):
    """Scale-const chain, replicated across `nparts` partitions. Writes vals
    [nparts,10]: cols 0..2 inv_i (=1/scale_i), 3..5 k_i (=sw_i*scale_i),
    6 mn, 7 MAGIC. Returns the list of final instructions (for dep hooks).

    scale_i = fl((mx-mn) * fl(1/qmax_i)): within ~1.5 ulp of the reference
    IEEE division; measured on the fixed inputs, the induced bucket flips
    change the final max rel err by < 1e-6 (the fp16 sel-minmax shift
    dominates), so the exact-residual Newton chain is not worth its 17
    serial steps on the critical path.
    """

    eng = eng if eng is not None else nc.vector
    P = nparts
    scale3 = tmp[0:P, 0:3]
    rng = tmp[0:P, 6:7]
    eng.tensor_sub(rng, scal_mx, scal_mn)
    eng.tensor_mul(scale3, rng.to_broadcast((P, 3)), y3)
    i1_ = nc.vector.reciprocal(vals[0:P, 0:3], scale3)
    i2 = eng.tensor_mul(vals[0:P, 3:6], scale3, sw)
    i3 = eng.tensor_copy(vals[0:P, 6:7], scal_mn)
    i4 = eng.memset(vals[0:P, 7:8], MAGIC)
    return [i1_, i2, i3, i4]


def _emit_quant(nc, pool, src, cbuf, nparts, nfree, out=None, sub_src=False, eng=None, sfx="", u_pre=None):
    """Emit the 3-bit blended fake-quant of src [nparts, nfree].

    u = src - mn
    r_i = u*inv_i + MAGIC          (two-stage TS op; the add rounds to
                                    integer RNE per ALU stage)
    p_i = (r_i - MAGIC) * k_i      (subtract exact, result = round(u/s)*k)
    result = p0 + p1 + p2 + mn     [- src if sub_src, giving the delta]
    Returns (output tile, instruction list).
    """
    eng = eng if eng is not None else nc.vector
    insts = []
    if u_pre is not None:
        u = u_pre
    else:
        u = pool.tile([nparts, nfree], F32, tag=f"qu_{nparts}_{nfree}{sfx}", name="qu")
        insts.append(eng.tensor_scalar(u, src, cbuf[:, 6:7], None, op0=ALU.subtract))
    p = []
    for i in range(3):
        # separate mul/add ops (not fused ACT): per-op IEEE fp32 rounding
        # must match the reference's separate ops, else near-tie elements
        # flip into the next quant bucket on HW
        pi = pool.tile(
            [nparts, nfree], F32, tag=f"ptmp{i}_{nparts}_{nfree}{sfx}", name=f"ptmp{i}"
        )
        insts.append(eng.tensor_scalar(
            pi, u, cbuf[:, i : i + 1], MAGIC, op0=ALU.mult, op1=ALU.add
        ))
        insts.append(eng.tensor_scalar(
            pi, pi, MAGIC, cbuf[:, 3 + i : 4 + i], op0=ALU.subtract, op1=ALU.mult
        ))
        p.append(pi)
    insts.append(eng.tensor_add(p[0], p[0], p[1]))
    insts.append(eng.tensor_add(p[0], p[0], p[2]))
    outt = out if out is not None else pool.tile(
        [nparts, nfree], F32, tag=f"qout_{nparts}_{nfree}{sfx}", name="qout"
    )
    if sub_src:
        # delta = (acc + mn) - src  (STT has no POOL opcode: always DVE)
        insts.append(nc.vector.scalar_tensor_tensor(
            outt, p[0], cbuf[:, 6:7], src, op0=ALU.add, op1=ALU.subtract
        ))
    else:
        insts.append(eng.tensor_scalar(outt, p[0], cbuf[:, 6:7], None, op0=ALU.add))
    return outt, insts


def _kernel_body(ctx, tc, ch, x_ap, selred_ap, selloc_ap, ws_ap, al_ap, out_ap, reps=1):
    nc = tc.nc

    const = ctx.enter_context(tc.tile_pool(name="const", bufs=1))
    rhs_pool = ctx.enter_context(tc.tile_pool(name="rhs", bufs=4))
    out_pool = ctx.enter_context(tc.tile_pool(name="outsb", bufs=4))
    psB = ctx.enter_context(tc.tile_pool(name="psB", bufs=4, space="PSUM"))

    # ---- inputs. selred chunks 0-2 lead the ACT queue, chunk 3 rides the
    # SP head so all four land by ~6us; the x stream follows on SP. ----
    alphas = const.tile([1, 6], F32)
    nc.gpsimd.dma_start(alphas[:], al_ap)  # SWDGE: off both HWDGE streams
    selredc = [
        const.tile([128, 2048], SELRED_DT, name=f"selredc{i}", tag=f"selredc{i}")
        for i in range(4)
    ]
    # W arrives pre-transposed from the host (quantization is elementwise,
    # so quant(W^T) == quant(W)^T): the quantized tiles ARE the lhsT
    # operands — no PE transposes, no PSUM staging, no identity matrix.
    # wcomb = [W^T k0 | W^T k1 | tiled W[:,ch]^T] in one [128,768] tile so
    # the whole weights path is one u-op + one split quant chain.
    wcomb = const.tile([128, 2 * C], F32)
    nc.sync.dma_start(wcomb[:], ws_ap)
    wtside = wcomb[:, 0 : 2 * C]
    nc.sync.dma_start(selredc[3][:], selred_ap[:, 3 * 2048 : 4 * 2048])
    for i in range(3):
        nc.scalar.dma_start(selredc[i][:], selred_ap[:, i * 2048 : (i + 1) * 2048])
    selloc = const.tile([128, 1024], F32)
    nc.scalar.dma_start(selloc[:], selloc_ap)

    # staging tile for the combined sel all-reduce: col0 = -min, col1 = max
    # (partition 0 holds the real max, rest -1e30)
    stage = const.tile([128, 2], F32)
    nc.gpsimd.memset(stage[:, 1:2], -1e30)

    with tc.high_priority():
        # ---- softmax of both alpha vectors (on partition 0), then one
        # early broadcast so both const chains can run 128-wide ----
        ex = const.tile([1, 6], F32)
        nc.scalar.activation(ex[:], alphas[:], ACTF.Exp)
        sums = const.tile([1, 8], F32)
        nc.vector.tensor_reduce(sums[0:1, 0:1], ex[0:1, 0:3], axis=AXIS.X, op=ALU.add)
        nc.vector.tensor_reduce(sums[0:1, 1:2], ex[0:1, 3:6], axis=AXIS.X, op=ALU.add)
        nc.vector.reciprocal(sums[0:1, 2:3], sums[0:1, 0:1])
        nc.vector.reciprocal(sums[0:1, 3:4], sums[0:1, 1:2])
        sw1 = const.tile([1, 6], F32)  # cols 0..2 = sw_activ, 3..5 = sw_weight
        nc.vector.tensor_scalar(sw1[0:1, 0:3], ex[0:1, 0:3], sums[0:1, 2:3], None, op0=ALU.mult)
        nc.vector.tensor_scalar(
            sw1[0:1, 3:6], ex[0:1, 3:6], sums[0:1, 3:4], None, op0=ALU.mult
        )
        sw = const.tile([128, 6], F32)
        nc.gpsimd.partition_broadcast(sw[:], sw1[0:1, :])

        # qmax and fl(1/qmax) constant vectors, replicated
        d3 = const.tile([128, 3], F32)
        y3 = const.tile([128, 3], F32)
        for i, qm in enumerate(QMAX):
            nc.gpsimd.memset(d3[:, i : i + 1], float(qm))
            nc.gpsimd.memset(y3[:, i : i + 1], float(np.float32(1.0) / np.float32(qm)))

        # ---- W min/max: DVE free-axis partials (runs before the sel data
        # lands), then a Pool all-reduce -> replicated scalars. The Pool
        # maxes are forced to wait for this all-reduce so the W-consts
        # chain starts early. ----
        wpart = const.tile([128, 2], F32)
        nc.vector.tensor_reduce(wpart[:, 0:1], wcomb[:, 0 : 2 * C], axis=AXIS.X, op=ALU.max)
        wminp = const.tile([128, 1], F32)
        nc.vector.tensor_reduce(wminp[:], wcomb[:, 0 : 2 * C], axis=AXIS.X, op=ALU.min)
        wneg_inst = nc.vector.tensor_scalar(wpart[:, 1:2], wminp[:], -1.0, None, op0=ALU.mult)
        gredw = const.tile([128, 2], F32)  # col0 wmx, col1 -wmn (replicated)
        arw_inst = nc.gpsimd.partition_all_reduce(
            gredw[:, 0:2], wpart[:, 0:2], channels=128, reduce_op=bass_isa.ReduceOp.max
        )
        wmn = const.tile([128, 1], F32)
        nc.vector.tensor_scalar(wmn[:], gredw[:, 1:2], -1.0, None, op0=ALU.mult)

        # ---- W consts (replicated on DVE), then one split quant chain:
        # Pool takes the lhsT 512 cols, ACT takes the 256 corr cols (its
        # fused scale*x+bias rounding differs from the reference only for
        # elements within one fp32 ulp of a .5 boundary - a few flips at
        # ~1e-3 rel each, inside the error budget) ----
        cbufw = const.tile([128, 10], F32)
        tmpw = const.tile([128, 40], F32)
        wconst_tail = _emit_scalar_consts(
            nc, cbufw, gredw[:, 0:1], wmn[:, 0:1], sw[:, 3:6], tmpw, d3, y3,
            nparts=128, eng=nc.vector,
        )
        uw = const.tile([128, 2 * C], F32)
        nc.gpsimd.tensor_scalar(uw[:], wcomb[:, 0 : 2 * C], cbufw[:, 6:7], None, op0=ALU.subtract)
        lhsT32 = const.tile([128, 2 * C], F32)
        lhsT = const.tile([128, 2 * C], F16)
        wq_chains = {}
        for mh in range(2):  # m0 cols on Pool (early); m1 cols on DVE (late,
            for k in range(2):  # after its activ half -- keeps Pool's queue
                c0 = k * C + mh * 128  # clear for wave drains)
                _, wq_chains[(mh, k)] = _emit_quant(
                    nc, const, wcomb[:, c0 : c0 + 128], cbufw, 128, 128,
                    out=lhsT32[:, c0 : c0 + 128],
                    eng=nc.gpsimd if mh == 0 else nc.vector, sfx=f"wq{mh}{k}",
                    u_pre=uw[:, c0 : c0 + 128],
                )
                nc.scalar.copy(lhsT[:, c0 : c0 + 128], lhsT32[:, c0 : c0 + 128])

    # ---- sel min/max: DVE takes the 4 min passes (plain min partials),
    # Pool takes the 4 all-axis max passes; one partition_all_reduce
    # replicates both so the consts chain runs 128-wide. The min passes
    # explicitly wait for the W consts chain (else the static schedule
    # interleaves 2.2us passes between its serial 60ns steps). ----
    sminp = const.tile([128, 4], F32)  # min partials
    sminc = const.tile([128, 1], F32)
    pmax = const.tile([1, 4], F32)  # per-chunk global maxes (partition 0)
    for i in range(4):
        mininst = nc.vector.tensor_reduce(
            sminp[:, i : i + 1], selredc[i][:], axis=AXIS.X, op=ALU.min
        )
        if i == 0:
            # both W minmax partials go first: they unlock AR_w -> W consts
            add_dep_helper(mininst.ins, wneg_inst.ins, reason="W partials first")
        maxinst = nc.gpsimd.tensor_reduce(
            pmax[0:1, i : i + 1], selredc[i][:], axis=AXIS.XYZWC, op=ALU.max
        )
        if i == 1:
            add_dep_helper(maxinst.ins, arw_inst.ins, reason="AR_w before c1max")
        if i == 2:
            lastmin_inst = mininst

    nc.vector.tensor_reduce(sminc[:], sminp[:, 0:4], axis=AXIS.X, op=ALU.min)
    nc.vector.tensor_scalar(stage[:, 0:1], sminc[:], -1.0, None, op0=ALU.mult)
    nc.vector.tensor_reduce(stage[0:1, 1:2], pmax[0:1, 0:4], axis=AXIS.X, op=ALU.max)
    gred = const.tile([128, 2], F32)  # col0 -smn, col1 smx (replicated)
    nc.gpsimd.partition_all_reduce(
        gred[:, 0:2], stage[:, 0:2], channels=128, reduce_op=bass_isa.ReduceOp.max
    )

    smn = const.tile([128, 1], F32)
    nc.vector.tensor_scalar(smn[:], gred[:, 0:1], -1.0, None, op0=ALU.mult)

    # ---- sel consts (replicated) + activ = blended fake-quant of sel;
    # cast to fp16 and scatter into the rhs k1 tiles (rows 120-127) ----
    cbufs = const.tile([128, 10], F32)
    tmps = const.tile([128, 40], F32)
    _emit_scalar_consts(
        nc, cbufs, gred[:, 1:2], smn[:, 0:1], sw[:, 0:3], tmps, d3, y3,
        nparts=128, eng=nc.vector,
    )
    activ16 = const.tile([128, 1024], F16)
    # quarter-split across DVE and Pool so the r=0 half (which gates the
    # first k1 matmuls via the scatter) finishes ~1us sooner; the final
    # +mn op writes fp16 directly (the scatter is the only consumer)
    _, ac_a = _emit_quant(
        nc, const, selloc[:, 0:320], cbufs, 128, 320,
        out=activ16[:, 0:320], eng=nc.vector, sfx="sa1",
    )
    _emit_quant(
        nc, const, selloc[:, 320:512], cbufs, 128, 192,
        out=activ16[:, 320:512], eng=nc.gpsimd, sfx="sa2",
    )
    _, ac_b = _emit_quant(
        nc, const, selloc[:, 512:832], cbufs, 128, 320,
        out=activ16[:, 512:832], eng=nc.vector, sfx="sb1",
    )
    _emit_quant(
        nc, const, selloc[:, 832:1024], cbufs, 128, 192,
        out=activ16[:, 832:1024], eng=nc.gpsimd, sfx="sb2",
    )
    # the m1 weight chains run after DVE's activ work (PE's m1 waves are
    # ~15us out; the activations gate every k1 main matmul)
    for k in range(2):
        add_dep_helper(wq_chains[(1, k)][0].ins, ac_b[-1].ins,
                       reason="activ before Wq-m1 on DVE")

    # ---- main loop. Scatter the quantized activations over the fp16 sel
    # rows of each k1 tile (selloc column r*512+s holds pixel r*2048+q*512+s,
    # so src/dst are clean 3D APs), then per (m-half, batch) wave: fill all
    # 8 PSUM banks with the K=256 mains, evict pairs to fp16 SBUF and DMA
    # out contiguous 2KB-per-partition chunks. All m=0 waves run before
    # m=1 so the m1 weight chains can quantize late without stalling PE. ----
    rhs = []
    for b in range(BPC):
        rhs0 = rhs_pool.tile([128, HW], F16, tag="rhs0")
        nc.sync.dma_start(rhs0[:], x_ap[b, 0:128, :])
        rhs1 = rhs_pool.tile([128, HW], F16, tag="rhs1")
        nc.sync.dma_start(rhs1[:], x_ap[b, 128:256, :])
        rhs.append((rhs0, rhs1))
    # pixel p = r*2048 + q*512 + s, so the r-half scatters are plain 2D
    # slices; gA's k1 matmuls (pixels 0:2048) only need the r=0 half, which
    # the DVE activ chain finishes first
    for b in range(BPC):
        for r in range(2):
            sc_eng = [nc.scalar, nc.gpsimd] if b < 2 else [nc.gpsimd, nc.gpsimd]
            for q in range(4):
                sc_eng[q % len(sc_eng)].dma_start(
                    rhs[b][1][120:128,
                              r * 2048 + q * 512 : r * 2048 + (q + 1) * 512],
                    activ16[b * 32 + q * 8 : b * 32 + (q + 1) * 8,
                            r * 512 : (r + 1) * 512],
                )

    # GPSIMD cannot access PSUM (BIR verifier) -> evicts on ACT/DVE only;
    # Pool carries extra out-writes instead
    ev_seq = ["D", "A", "D", "D", "A", "D", "A", "D"] + ["D", "A"] * 28
    wq_seq = ["A", "P", "A", "P", "A", "P", "P", "A",
              "S", "P", "S", "A", "P", "S", "P", "S",
              "A", "P", "S", "P", "S", "A", "P", "S",
              "P", "P", "S", "P"]
    ev_map = {"D": nc.vector.tensor_copy, "A": nc.scalar.copy}
    wq_map = {"S": nc.sync, "A": nc.scalar, "P": nc.gpsimd}
    ev_i = 0
    wq_i = 0
    # ---- PE clock warmup: ~20 discarded matmuls (into wave 1's first
    # PSUM pair, overwritten by its real start=True mains) keep the PE
    # busy from ~12us so the real stream runs at full clock with no ramp.
    # Gated on the last sel min pass so they don't run at t=0 and idle out.
    pair00 = psB.tile([128, 1024], F32, name="ptile", tag="ptile")
    for wi in range(14):
        wmm = nc.tensor.matmul(
            pair00[:, 0:512],
            selredc[0][:, 0:128],
            selredc[0][:, 512:1024],
            start=True,
            stop=True,
            skip_group_check=True,
        )
        if wi == 0:
            add_dep_helper(wmm.ins, lastmin_inst.ins, reason="warmup after mins")
    first_mm = [None]
    for rep in range(reps):
        for m in range(2):
            for b in range(BPC):
                rhs0, rhs1 = rhs[b]
                groups = ([0, 1, 2, 3], [4, 5, 6, 7])  # contiguous pixels
                # [128,1024] PSUM tiles (2 banks each): matmuls hit 512-wide
                # slices, the evict reads the pair in one op
                pairs = {}
                for ns in groups:
                    for n in ns:
                        if n % 2 == 0:
                            if rep == 0 and m == 0 and b == 0 and n == 0:
                                pairs[0] = pair00
                            else:
                                pairs[n // 2] = psB.tile(
                                    [128, 1024], F32, name="ptile", tag="ptile"
                                )
                        mm = nc.tensor.matmul(
                            pairs[n // 2][:, (n % 2) * 512 : (n % 2 + 1) * 512],
                            lhsT[:, m * 128 : (m + 1) * 128],
                            rhs0[:, n * 512 : (n + 1) * 512],
                            start=True,
                            stop=False,
                        )

                    for n in ns:
                        nc.tensor.matmul(
                            pairs[n // 2][:, (n % 2) * 512 : (n % 2 + 1) * 512],
                            lhsT[:, C + m * 128 : C + (m + 1) * 128],
                            rhs1[:, n * 512 : (n + 1) * 512],
                            start=False,
                            stop=True,
                        )
                outsb = out_pool.tile([128, HW], F16, name="outsb", tag="outsb")
                is_last = m == 1 and b >= BPC - 2 and rep == reps - 1
                for gi, ns in enumerate(groups):
                    if is_last:
                        # final drain: per-512 chunks fanned across engines
                        # and queues so the tail after the last matmul is
                        # one small copy + one small DMA
                        levs = [nc.scalar.copy, nc.vector.tensor_copy,
                                nc.scalar.copy, nc.vector.tensor_copy]
                        lwqs = ([nc.scalar, nc.sync, nc.gpsimd, nc.sync]
                                if b == BPC - 1 else
                                [nc.gpsimd, nc.sync, nc.scalar, nc.gpsimd])
                        for j, n in enumerate(ns):
                            levs[j](
                                outsb[:, n * 512 : (n + 1) * 512],
                                pairs[n // 2][:, (n % 2) * 512 : (n % 2 + 1) * 512],
                            )
                            lwqs[j].dma_start(
                                out_ap[b, m * 128 : (m + 1) * 128,
                                       n * 512 : (n + 1) * 512],
                                outsb[:, n * 512 : (n + 1) * 512],
                            )
                        continue
                    # paired evicts (2 x [128,1024]), each written out as
                    # soon as it lands (pipelines the drain within a group)
                    for half in range(2):
                        pr = ns[0] // 2 + half
                        ev_map[ev_seq[ev_i % len(ev_seq)]](
                            outsb[:, pr * 1024 : (pr + 1) * 1024], pairs[pr][:]
                        )
                        ev_i += 1
                        wq_map[wq_seq[wq_i % len(wq_seq)]].dma_start(
                            out_ap[b, m * 128 : (m + 1) * 128,
                                   pr * 1024 : (pr + 1) * 1024],
                            outsb[:, pr * 1024 : (pr + 1) * 1024],
                        )
                        wq_i += 1


def build_program(ch, reps=1):
    nc = bacc.Bacc(
        "TRN2", target_bir_lowering=False, debug=False, num_devices=NCORES
    )
    x_t = nc.dram_tensor("x", [BPC, C, HW], F16, kind="ExternalInput").ap()
    selred_t = nc.dram_tensor("selred", [128, 8192], SELRED_DT, kind="ExternalInput").ap()
    selloc_t = nc.dram_tensor("selloc", [128, 1024], F32, kind="ExternalInput").ap()
    ws_t = nc.dram_tensor("wselt", [128, 2 * C], F32, kind="ExternalInput").ap()
    al_t = nc.dram_tensor("alphas", [1, 6], F32, kind="ExternalInput").ap()
    out_t = nc.dram_tensor("out", [BPC, C, HW], F16, kind="ExternalOutput").ap()
    with tile.TileContext(nc) as tc:
        with ExitStack() as ctx:
            _kernel_body(
                ctx, tc, ch, x_t, selred_t, selloc_t, ws_t, al_t, out_t,
                reps=reps,
            )
    nc.compile()
    return nc


def make_in_maps(x, alpha_activ, alpha_weight, conv_weight, selected_channels):
    x = np.ascontiguousarray(np.asarray(x, dtype=np.float32).reshape(B, C, HW))
    ch = [int(v) for v in np.asarray(selected_channels).ravel()]
    sel = np.ascontiguousarray(x[:, ch, :])  # [32, 8, 4096]
    # channel permutation: the 8 selected channels go LAST (k1 rows 120-127)
    # so the quantized activations scatter into the rhs tiles as one
    # contiguous partition block; permuting x's channels and W's columns
    # identically leaves the conv output unchanged
    perm = [c for c in range(C) if c not in set(ch)] + ch
    selred_np = np.float16 if SELRED_F16 else np.float32
    selred = sel.reshape(128, 8192).astype(selred_np)
    x16 = x[:, perm, :].astype(np.float16)
    alphas = np.concatenate(
        [np.asarray(alpha_activ).ravel(), np.asarray(alpha_weight).ravel()]
    ).astype(np.float32).reshape(1, 6)
    wmat = np.asarray(conv_weight, dtype=np.float32).reshape(C, C)
    wt = wmat.T[perm, :]  # rows follow the channel permutation
    # cols 0:512 = permuted W^T k-chunks side by side
    wselt = np.concatenate([wt[0:128, :], wt[128:256, :]], axis=1).astype(np.float32)
    wselt = np.ascontiguousarray(wselt)
    in_maps = []
    for c in range(NCORES):
        xs = np.ascontiguousarray(x16[c * BPC : (c + 1) * BPC])
        # selloc layout: partition p = b*32 + q*8 + j, col r*512+s holds
        # sel[core*4+b, j, r*2048 + q*512 + s] -- so PSUM group g covers the
        # contiguous pixel range [g*2048, (g+1)*2048) yet needs only delta
        # column-half g
        sl = sel[c * BPC : (c + 1) * BPC].reshape(BPC, NSEL, 2, 4, 512)
        selloc = np.ascontiguousarray(
            sl.transpose(0, 3, 1, 2, 4).reshape(128, 1024)
        )
        in_maps.append(
            {
                "x": xs,
                "selred": selred,
                "selloc": selloc,
                "wselt": wselt,
                "alphas": alphas,
            }
        )
    return ch, in_maps


def kernel(x, alpha_activ, alpha_weight, conv_weight, selected_channels):
    from concourse.bass_utils import run_bass_kernel_spmd

    ch, in_maps = make_in_maps(
        x, alpha_activ, alpha_weight, conv_weight, selected_channels
    )
    nc = build_program(ch)
    res = run_bass_kernel_spmd(nc, in_maps, core_ids=list(range(NCORES)))
    outs = [
        res.results[c]["out"].astype(np.float32).reshape(BPC, C, H, W)
        for c in range(NCORES)
    ]
    return np.concatenate(outs, axis=0)
